# revision 2
# baseline (speedup 1.0000x reference)
"""Trainium2 Bass kernel for nn_DependencyParser.

SPMD over 8 NeuronCores; cores 0-3 run the forward LSTM direction, cores 4-7
the backward direction -- one identical program, direction expressed purely
through per-core DATA (time-reversed gather indices, direction-specific
weights, 0/1 orientation masks).  Per core:
  - on-device embedding gather (dma_gather; paired-row trick since the 50000
    vocab exceeds the int16 index range), PE-transpose to [feature, time]
  - 2-layer LSTM via block Gauss-Seidel sweeps over the hidden sequence;
    each sweep is a batched [2048,512]x[512,512] matmul + gate activations;
    the cell state is computed EXACTLY per sweep with the hardware linear
    scan op (tensor_tensor_scan).  Every wave is processed in two 256-col
    TIME CHUNKS: the scan chains across chunks via an AP initial value, and
    the next wave's dependent matmuls are issued per (kt, chunk) in oldest-
    dependency-first order, so the serial scan->tanh->mult tail is hidden
    under independent matmul work and the PE never idles long enough for
    the HAM clock gate to re-throttle.
  - after each layer the two directions exchange hidden states with TWO
    pairwise AllGathers (h01 fired right after wave 0 of the final sweep,
    h23 after wave 1) so the collective latency overlaps the final sweep
    tail + the next layer's local input pass.
  - head MLP, then pairwise scores tanh(mlp[i]+mlp[j]) @ out_w for 64 head
    "slots".  The pair function is symmetric in (head, child), so each head
    only computes a 264-wide cyclic window of children (mirror coverage
    halves the tanh work); the host reconstructs the full [L, L-1] matrix.
    tanh is batched 4 slots per ACTIVATE; the mt=3 broadcast-adds run on
    the otherwise-idle GpSimd engine.
Matmul operands are fp16 (1 cyc/row); PRE = W_ih@x + b is stored bf16; the
g/o gate PRE is injected into the gate PSUM via identity-matmuls (TensorE),
the i/f gate PRE via DVE adds, balancing PE vs DVE per wave.  Gate tiles are
fp16 so the cell-state chain runs in DVE 2x mode.  L0's LSTM bias rides a
ones-row in the x matmul.  Bulk weights for layer 1 / MLP are DMA'd after
the L0 sweeps start so the embedding gather gets full HBM bandwidth.
"""
import sys
sys.path.insert(0, '/opt/trn_rl_repo')
import numpy as np

import concourse.bass as bass
import concourse.mybir as mybir
import concourse.tile as tile
from concourse import bacc
from concourse.masks import make_identity
from concourse.bass_utils import run_bass_kernel_spmd

F32 = mybir.dt.float32
F16 = mybir.dt.float16
BF16 = mybir.dt.bfloat16
I16 = mybir.dt.int16
AF = mybir.ActivationFunctionType
OP = mybir.AluOpType

L = 512
NG = 2048
V2 = 25000
WD, PD = 256, 64
DIN = WD + PD
M = 512
NCORES = 8
HPC = L // NCORES
WLEN = 264           # pairwise child window (cyclic, via doubled child axis)
DBL = 768            # doubled child axis length actually needed (8*63+264)
K0 = 6
K1 = 7

CHS = ((0, 256), (256, 512))        # time chunks within a wave
INJECT = {4, 5, 6, 7}               # g/o pairs: pre via TensorE identity-matmul

_CACHE = {}


def _head_of(c, j):
    if c < 4:
        return 8 * j + 4 + c
    return 8 * ((31 - j) % 64) + (c - 4)


def _child_of_vec(c, j):
    i = np.arange(WLEN)
    if c < 4:
        return (8 * j + i) % 512
    return (511 - 8 * j - i) % 512


def _emit_sweeps(nc, wp, gp, psum, identr, pre, whh, hA, hB, n_sweeps,
                 on_final=None):
    """Block Gauss-Seidel sweeps, chunked in time.  hA/hB are (tile01,
    tile23) double buffers holding the SHIFTED hidden sequence
    (storage[t] = h_{t-1}).  Wave 2 (hidden dims 256:512) consumes wave 1's
    fresh h01 from the same sweep.  Returns (fin01, fin23) holding the
    UNSHIFTED final h, written into the retiring hprev buffers; on_final(wv,
    tile) is invoked right after each final-sweep wave's unshifted write so
    the caller can overlap collectives with the remaining wave."""
    sig, tanh = AF.Sigmoid, AF.Tanh
    G = {}
    for g in range(4):
        G[g] = gp.tile([128, 4, L], F16, tag=f"G{g}", name=f"G{g}")

    for s in range(n_sweeps):
        hprev, hnew = (hA, hB) if s % 2 == 0 else (hB, hA)
        last = s == n_sweeps - 1
        for wv in range(2):
            hs = 2 * wv
            # pair order i, g, f, o within each chunk: the o gate finishes
            # last so the cell chain completes during the o matmuls.
            pairs = (0, 4, 2, 6) if wv == 0 else (1, 5, 3, 7)
            # oldest-dependency kt first: wave0's freshest input is the
            # previous wave's h23; wave1's is this sweep's fresh h01.
            ktorder = (0, 1, 2, 3) if wv == 0 else (2, 3, 0, 1)
            zps = {}
            if s > 0:
                for a in pairs:
                    zps[a] = psum.tile([128, 2, L], F32, tag="zp",
                                       name=f"zp{s}_{a}")
            u = wp.tile([128, 2, L], F16, tag="u", name=f"u{s}_{wv}")
            cst = wp.tile([128, 2, L], F32, tag="c", name=f"c{s}_{wv}")
            tc_ = wp.tile([128, 2, L], F16, tag="tc", name=f"tc{s}_{wv}")
            for ci, (c0, c1) in enumerate(CHS):
                if s > 0:
                    for kt in ktorder:
                        src = hnew if (wv == 1 and kt < 2) else hprev
                        rhs = src[kt // 2][:, kt % 2, c0:c1]
                        for a in pairs:
                            for half in range(2):
                                nt = 2 * a + half
                                nc.tensor.matmul(
                                    zps[a][:, half, c0:c1],
                                    whh[:, kt, nt * 128:(nt + 1) * 128],
                                    rhs, start=(kt == ktorder[0]),
                                    stop=(kt == ktorder[3] and
                                          a not in INJECT),
                                    skip_group_check=(ci == 1))
                    for a in pairs:
                        if a in INJECT:
                            for half in range(2):
                                nt = 2 * a + half
                                nc.tensor.matmul(zps[a][:, half, c0:c1],
                                                 identr[:],
                                                 pre[:, nt, c0:c1],
                                                 start=False, stop=True,
                                                 skip_group_check=(ci == 1))
                # gate activations for this chunk (i, g, f, o)
                for a in pairs:
                    gate = (2 * a) // 4
                    fn = tanh if gate == 2 else sig
                    if s == 0:
                        nc.scalar.activation(G[gate][:, hs:hs + 2, c0:c1],
                                             pre[:, 2 * a:2 * a + 2, c0:c1],
                                             fn)
                    else:
                        if a not in INJECT:
                            nc.vector.tensor_tensor(
                                zps[a][:, :, c0:c1], zps[a][:, :, c0:c1],
                                pre[:, 2 * a:2 * a + 2, c0:c1], OP.add)
                        nc.scalar.activation(G[gate][:, hs:hs + 2, c0:c1],
                                             zps[a][:, :, c0:c1], fn)
                # cell chain for this chunk
                nc.vector.tensor_tensor(u[:, :, c0:c1],
                                        G[0][:, hs:hs + 2, c0:c1],
                                        G[2][:, hs:hs + 2, c0:c1], OP.mult)
                for b in range(2):
                    init = 0.0 if ci == 0 else cst[:, b, c0 - 1:c0]
                    nc.vector.tensor_tensor_scan(
                        cst[:, b, c0:c1], G[1][:, hs + b, c0:c1],
                        u[:, b, c0:c1], init, OP.mult, OP.add)
                nc.scalar.activation(tc_[:, :, c0:c1], cst[:, :, c0:c1], tanh)
                # shifted store (hnew[t+1] = h_t); the last column drops out
                se = min(c1, L - 1)
                if not last or wv == 0:
                    nc.vector.tensor_tensor(hnew[wv][:, :, c0 + 1:se + 1],
                                            G[3][:, hs:hs + 2, c0:se],
                                            tc_[:, :, c0:se], OP.mult)
                if last:
                    nc.vector.tensor_tensor(hprev[wv][:, :, c0:c1],
                                            G[3][:, hs:hs + 2, c0:c1],
                                            tc_[:, :, c0:c1], OP.mult)
            if last and on_final is not None:
                on_final(wv, hprev[wv])
    return hA if (n_sweeps - 1) % 2 == 0 else hB


def _build_program():
    nc = bacc.Bacc("TRN2", target_bir_lowering=False, debug=False,
                   num_devices=NCORES)

    def dram_in(name, shape, dtype=F32):
        return nc.dram_tensor(name, shape, dtype, kind="ExternalInput")

    w2_d = dram_in("w2", [V2, 2 * WD])
    pemb_d = dram_in("pemb", [50, PD])
    widx_d = dram_in("widx", [128, 32], I16)
    pidx_d = dram_in("pidx", [128, 32], I16)
    wpar_d = dram_in("wpar", [128, 4, 1])
    wih0_d = dram_in("wih0", [128, 3, NG], F16)   # bias folded in row 320
    whh0_d = dram_in("whh0", [128, 4, NG], F16)
    wihl_d = dram_in("wih1loc", [2, 128, 4, NG // 2], F16)
    wihr_d = dram_in("wih1rem", [2, 128, 4, NG // 2], F16)
    whh1_d = dram_in("whh1", [128, 4, NG], F16)
    b1_d = dram_in("b1", [128, 16])
    mlpwl_d = dram_in("mlpwloc", [128, 4, M], F16)
    mlpwr_d = dram_in("mlpwrem", [128, 4, M], F16)
    mlpb2_d = dram_in("mlpb2", [128, 4])
    outw_d = dram_in("outw", [128, 4, 128], F16)
    sel_d = dram_in("sel", [128, 4, HPC], F16)
    mi_d = dram_in("maskI", [128, 1])
    mr_d = dram_in("maskR", [128, 1])
    out_d = nc.dram_tensor("out", [HPC, WLEN], F32, kind="ExternalOutput")

    GROUPS = [[0, 4], [1, 5], [2, 6], [3, 7]]

    with tile.TileContext(nc) as tc:
        with (
            tc.tile_pool(name="pp", bufs=1) as pp,
            tc.tile_pool(name="wp", bufs=2) as wp,
            tc.tile_pool(name="gp", bufs=1) as gp,
            tc.tile_pool(name="dram", bufs=2, space="DRAM") as dp,
        ):
            ident = pp.tile([128, 128], F32, tag="ident")
            make_identity(nc, ident[:])
            identr = pp.tile([128, 128], BF16, tag="identr")
            nc.vector.tensor_copy(identr[:], ident[:])
            identh = pp.tile([128, 128], F16, tag="identh")
            nc.vector.tensor_copy(identh[:], ident[:])
            zsrc = pp.tile([128, 1], F32, tag="zsrc")
            nc.vector.memset(zsrc[:], 0.0)
            mI = pp.tile([128, 1], F32, tag="mI")
            mR = pp.tile([128, 1], F32, tag="mR")
            nc.sync.dma_start(mI[:], mi_d[:])
            nc.sync.dma_start(mR[:], mr_d[:])
            xrem = pp.tile([128, 4, L], F16, tag="xrem")
            mlpwl = pp.tile([128, 4, M], F16, tag="mlpwl")
            mlpwr = pp.tile([128, 4, M], F16, tag="mlpwr")
            sel = pp.tile([128, 4, HPC], F16, tag="sel")
            mlpb2 = pp.tile([128, 4], F32, tag="mlpb2")
            outw = pp.tile([128, 4, 128], F16, tag="outw")
            hA = tuple(pp.tile([128, 2, L], F16, tag=f"hA{i}", name=f"hA{i}")
                       for i in range(2))
            hB = tuple(pp.tile([128, 2, L], F16, tag=f"hB{i}", name=f"hB{i}")
                       for i in range(2))
            for t in hA + hB:
                nc.vector.tensor_copy(t[:, :, 0:1],
                                      zsrc[:, 0:1].to_broadcast([128, 2, 1]))
            s0 = pp.tile([128, 4, L], F16, tag="slot0")
            s1 = pp.tile([128, 4, L], F16, tag="slot1")

            def build_xrem_half(slot0, slot1, kts):
                # remote direction's hidden, re-oriented to my time order
                for kt in kts:
                    tmp = wp.tile([128, L], F32, tag="u", name=f"xt{kt}")
                    nc.vector.tensor_scalar_mul(tmp[:], slot0[:, kt, ::-1],
                                                mR[:])
                    nc.vector.scalar_tensor_tensor(
                        xrem[:, kt, :], slot1[:, kt, ::-1], mI[:], tmp[:],
                        OP.mult, OP.add)

            with tc.tile_pool(name="post", bufs=1) as post:
              mlp_tm = post.tile([128, 4, M], F16, tag="mlp_tm")
              mlpD = post.tile([128, 4, DBL], F16, tag="mlpD")
              myb = post.tile([128, 4, HPC], F32, tag="myb")
              with tc.tile_pool(name="psA", bufs=4, space="PSUM") as psum:
                with (
                    tc.tile_pool(name="prep", bufs=1) as prep,
                    tc.tile_pool(name="wts", bufs=1) as wts,
                ):
                    pre0 = prep.tile([128, 16, L], BF16, tag="pre0")
                    pre1 = prep.tile([128, 16, L], BF16, tag="pre1")
                    bT1 = prep.tile([128, 16], F32, tag="bT1")

                    # ---------- gather + transpose ----------
                    # index DMAs + gathers issued FIRST; the bulk layer-1/MLP
                    # weights are deferred until the L0 sweeps are emitted so
                    # the gather gets full HBM bandwidth.
                    xT = prep.tile([128, 3, L], F16, tag="xT")
                    with tc.tile_pool(name="pC", bufs=1) as pC:
                        widx = pC.tile([128, 32], I16, tag="widx")
                        pidx = pC.tile([128, 32], I16, tag="pidx")
                        wpar = pC.tile([128, 4, 1], F32, tag="wpar")
                        nc.sync.dma_start(widx[:], widx_d[:])
                        nc.sync.dma_start(pidx[:], pidx_d[:])
                        nc.sync.dma_start(wpar[:], wpar_d[:])
                        pair = pC.tile([128, 4, 2 * WD], F32, tag="pair")
                        nc.gpsimd.dma_gather(pair[:], w2_d[:], widx[:], L, L,
                                             elem_size=2 * WD)
                        xp = pC.tile([128, 4, PD], F32, tag="xp")
                        nc.gpsimd.dma_gather(xp[:], pemb_d[:], pidx[:], L, L,
                                             elem_size=PD)
                        # layer-0 weights (needed first)
                        wih0 = wts.tile([128, 3, NG], F16, tag="wih0")
                        nc.sync.dma_start(wih0[:], wih0_d[:])
                        whh0 = wts.tile([128, 4, NG], F16, tag="whh0")
                        nc.sync.dma_start(whh0[:], whh0_d[:])
                        x = pC.tile([128, 4, DIN], F32, tag="x")
                        xw = x[:, :, 0:WD]
                        nc.vector.tensor_tensor(xw, pair[:, :, WD:2 * WD],
                                                pair[:, :, 0:WD], OP.subtract)
                        nc.vector.tensor_tensor(
                            xw, xw, wpar[:].to_broadcast([128, 4, WD]),
                            OP.mult)
                        nc.vector.tensor_tensor(xw, xw, pair[:, :, 0:WD],
                                                OP.add)
                        nc.vector.tensor_copy(x[:, :, WD:DIN], xp[:])
                        # zero pad rows, then the ones bias row (din 320)
                        nc.vector.tensor_copy(
                            xT[64:128, 2, :],
                            zsrc[64:128, 0:1].to_broadcast([64, L]))
                        nc.vector.memset(xT[64:65, 2, :], 1.0)
                        for ct in range(4):
                            for dblk, wdt in ((0, 128), (1, 128), (2, 64)):
                                tp = psum.tile([128, 2, L], F32, tag="zp",
                                               name=f"tp{ct}_{dblk}")
                                nc.tensor.transpose(
                                    tp[0:wdt, 0, 0:128],
                                    x[:, ct, dblk * 128:dblk * 128 + wdt],
                                    ident[:])
                                nc.vector.tensor_copy(
                                    xT[0:wdt, dblk, ct * 128:(ct + 1) * 128],
                                    tp[0:wdt, 0, 0:128])

                    # ------- layer 0 pre (bias folded into matmul) -------
                    for a in (0, 4, 2, 6, 1, 5, 3, 7):
                        zp = psum.tile([128, 2, L], F32, tag="zp",
                                       name=f"p0_{a}")
                        for kt in range(3):
                            for half in range(2):
                                nt = 2 * a + half
                                nc.tensor.matmul(
                                    zp[:, half, :],
                                    wih0[:, kt, nt * 128:(nt + 1) * 128],
                                    xT[:, kt, :], start=(kt == 0),
                                    stop=(kt == 2))
                        nc.vector.tensor_copy(pre0[:, 2 * a:2 * a + 2, :],
                                              zp[:])

                    # deferred bulk weights (layer 1 + MLP + pairwise)
                    wl = []
                    wr = []
                    for nh in range(2):
                        t = wts.tile([128, 4, NG // 2], F16, tag=f"wl{nh}")
                        nc.sync.dma_start(t[:], wihl_d[nh])
                        wl.append(t)
                    for nh in range(2):
                        t = wts.tile([128, 4, NG // 2], F16, tag=f"wr{nh}")
                        nc.sync.dma_start(t[:], wihr_d[nh])
                        wr.append(t)
                    whh1 = wts.tile([128, 4, NG], F16, tag="whh1")
                    nc.sync.dma_start(whh1[:], whh1_d[:])
                    nc.sync.dma_start(bT1[:], b1_d[:])
                    nc.sync.dma_start(mlpwl[:], mlpwl_d[:])
                    nc.sync.dma_start(mlpwr[:], mlpwr_d[:])
                    nc.sync.dma_start(sel[:], sel_d[:])
                    nc.sync.dma_start(mlpb2[:], mlpb2_d[:])
                    nc.sync.dma_start(outw[:], outw_d[:])

                    # ---------- layer 0 sweeps (split collective) ----------
                    ag0 = {}

                    def fire_ag0(wv, htile):
                        inb = dp.tile([128, 2, L], F16, tag=f"inb{wv}",
                                      name=f"inb0_{wv}")
                        outb = dp.tile([2, 128, 2, L], F16, tag=f"outb{wv}",
                                       name=f"outb0_{wv}")
                        nc.sync.dma_start(inb[:], htile[:])
                        nc.gpsimd.collective_compute(
                            "AllGather", OP.bypass, replica_groups=GROUPS,
                            ins=[inb.opt()], outs=[outb.opt()])
                        ag0[wv] = outb

                    hfin0 = _emit_sweeps(nc, wp, gp, psum, identr, pre0,
                                         whh0, hA, hB, K0, on_final=fire_ag0)

                    # ------- layer 1 pre: local pass (overlaps collective) --
                    def pass1_pair(a):
                        zp = psum.tile([128, 2, L], F32, tag="zp",
                                       name=f"p1_{a}")
                        nh = (2 * a) // 8
                        for kt in range(4):
                            for half in range(2):
                                nt = 2 * a + half
                                off = (nt - nh * 8) * 128
                                nc.tensor.matmul(
                                    zp[:, half, :],
                                    wl[nh][:, kt, off:off + 128],
                                    hfin0[kt // 2][:, kt % 2, :],
                                    start=(kt == 0), stop=(kt == 3))
                        for half in range(2):
                            nt = 2 * a + half
                            nc.scalar.activation(pre1[:, nt, :],
                                                 zp[:, half, :], AF.Identity,
                                                 bias=bT1[:, nt:nt + 1])

                    for a in (0, 4, 2, 6, 1, 5, 3, 7):
                        pass1_pair(a)
                    # first half of the exchanged hidden -> xrem kt 0/1
                    nc.sync.dma_start(s0[:, 0:2, :], ag0[0][0])
                    nc.sync.dma_start(s1[:, 0:2, :], ag0[0][1])
                    build_xrem_half(s0, s1, (0, 1))
                    # remote pass, accumulated into pre1 via identity-inject;
                    # two batches of 4 pairs, kt 0/1 first so the second
                    # AllGather half has time to land.
                    for batch in ((0, 4, 2, 6), (1, 5, 3, 7)):
                        zpb = {}
                        for a in batch:
                            zpb[a] = psum.tile([128, 2, L], F32, tag="zp",
                                               name=f"p2_{a}")
                            nh = (2 * a) // 8
                            for half in range(2):
                                nt = 2 * a + half
                                nc.tensor.matmul(zpb[a][:, half, :],
                                                 identr[:], pre1[:, nt, :],
                                                 start=True, stop=False)
                            for kt in (0, 1):
                                for half in range(2):
                                    nt = 2 * a + half
                                    off = (nt - nh * 8) * 128
                                    nc.tensor.matmul(
                                        zpb[a][:, half, :],
                                        wr[nh][:, kt, off:off + 128],
                                        xrem[:, kt, :], start=False,
                                        stop=False)
                        if batch[0] == 0:
                            nc.sync.dma_start(s0[:, 2:4, :], ag0[1][0])
                            nc.sync.dma_start(s1[:, 2:4, :], ag0[1][1])
                            build_xrem_half(s0, s1, (2, 3))
                        for a in batch:
                            nh = (2 * a) // 8
                            for kt in (2, 3):
                                for half in range(2):
                                    nt = 2 * a + half
                                    off = (nt - nh * 8) * 128
                                    nc.tensor.matmul(
                                        zpb[a][:, half, :],
                                        wr[nh][:, kt, off:off + 128],
                                        xrem[:, kt, :], start=False,
                                        stop=(kt == 3))
                        for a in batch:
                            nc.scalar.activation(pre1[:, 2 * a:2 * a + 2, :],
                                                 zpb[a][:], AF.Copy)

                    # ------- layer 1 sweeps (reuse the L0 h buffers) -------
                    for t in hfin0:
                        nc.vector.tensor_copy(
                            t[:, :, 0:1],
                            zsrc[:, 0:1].to_broadcast([128, 2, 1]))
                    ag1 = {}

                    def fire_ag1(wv, htile):
                        inb = dp.tile([128, 2, L], F16, tag=f"inb{wv}",
                                      name=f"inb1_{wv}")
                        outb = dp.tile([2, 128, 2, L], F16, tag=f"outb{wv}",
                                       name=f"outb1_{wv}")
                        nc.sync.dma_start(inb[:], htile[:])
                        nc.gpsimd.collective_compute(
                            "AllGather", OP.bypass, replica_groups=GROUPS,
                            ins=[inb.opt()], outs=[outb.opt()])
                        ag1[wv] = outb

                    hfin1 = _emit_sweeps(nc, wp, gp, psum, identr, pre1,
                                         whh1, hA, hB, K1, on_final=fire_ag1)

                # ---------- MLP (into mlpD[:, :, 0:L] and mlp_tm) ----------
                for mp in (0, 2):
                    zp = psum.tile([128, 2, L], F32, tag="zp", name=f"mm{mp}")
                    for kt in range(4):
                        for half in range(2):
                            mt = mp + half
                            nc.tensor.matmul(
                                zp[:, half, :],
                                mlpwl[:, kt, mt * 128:(mt + 1) * 128],
                                hfin1[kt // 2][:, kt % 2, :],
                                start=(kt == 0), stop=(kt == 3))
                    nc.scalar.activation(mlpD[:, mp:mp + 2, 0:L], zp[:],
                                         AF.Copy)
                for tq in (0, 2):
                    zp = psum.tile([128, 2, L], F32, tag="zp", name=f"mt{tq}")
                    for kt in range(4):
                        for half in range(2):
                            tt = tq + half
                            nc.tensor.matmul(
                                zp[:, half, :],
                                hfin1[kt // 2][:, kt % 2, tt * 128:(tt + 1) * 128],
                                mlpwl[:, kt, :], start=(kt == 0),
                                stop=(kt == 3))
                    nc.scalar.activation(mlp_tm[:, tq:tq + 2, :], zp[:],
                                         AF.Copy)
                nc.sync.dma_start(s0[:, 0:2, :], ag1[0][0])
                nc.sync.dma_start(s1[:, 0:2, :], ag1[0][1])
                build_xrem_half(s0, s1, (0, 1))
                # remote halves accumulated via identity-inject; kt 0/1
                # phase first across all four groups, then kt 2/3.
                zr = {}
                for mp in (0, 2):
                    zr[('m', mp)] = psum.tile([128, 2, L], F32, tag="zp",
                                              name=f"rm{mp}")
                    zr[('t', mp)] = psum.tile([128, 2, L], F32, tag="zp",
                                              name=f"rt{mp}")
                for mp in (0, 2):
                    zp = zr[('m', mp)]
                    for half in range(2):
                        mt = mp + half
                        nc.tensor.matmul(zp[:, half, :], identh[:],
                                         mlpD[:, mt, 0:L], start=True,
                                         stop=False)
                    for kt in (0, 1):
                        for half in range(2):
                            mt = mp + half
                            nc.tensor.matmul(
                                zp[:, half, :],
                                mlpwr[:, kt, mt * 128:(mt + 1) * 128],
                                xrem[:, kt, :], start=False, stop=False)
                for tq in (0, 2):
                    zp = zr[('t', tq)]
                    for half in range(2):
                        tt = tq + half
                        nc.tensor.matmul(zp[:, half, :], identh[:],
                                         mlp_tm[:, tt, :], start=True,
                                         stop=False)
                    for kt in (0, 1):
                        for half in range(2):
                            tt = tq + half
                            nc.tensor.matmul(
                                zp[:, half, :],
                                xrem[:, kt, tt * 128:(tt + 1) * 128],
                                mlpwr[:, kt, :], start=False, stop=False)
                nc.sync.dma_start(s0[:, 2:4, :], ag1[1][0])
                nc.sync.dma_start(s1[:, 2:4, :], ag1[1][1])
                build_xrem_half(s0, s1, (2, 3))
                for mp in (0, 2):
                    zp = zr[('m', mp)]
                    for kt in (2, 3):
                        for half in range(2):
                            mt = mp + half
                            nc.tensor.matmul(
                                zp[:, half, :],
                                mlpwr[:, kt, mt * 128:(mt + 1) * 128],
                                xrem[:, kt, :], start=False, stop=(kt == 3))
                    nc.scalar.activation(mlpD[:, mp:mp + 2, 0:L], zp[:],
                                         AF.Copy)
                for tq in (0, 2):
                    zp = zr[('t', tq)]
                    for kt in (2, 3):
                        for half in range(2):
                            tt = tq + half
                            nc.tensor.matmul(
                                zp[:, half, :],
                                xrem[:, kt, tt * 128:(tt + 1) * 128],
                                mlpwr[:, kt, :], start=False, stop=(kt == 3))
                    nc.scalar.activation(mlp_tm[:, tq:tq + 2, :], zp[:],
                                         AF.Copy)
                # head-slot mlp vectors (one-hot select over time) + 2*bias
                for mt in range(4):
                    zp = psum.tile([128, 2, L], F32, tag="zp", name=f"my{mt}")
                    zv = zp[:, 0, 0:HPC]
                    for tt in range(4):
                        nc.tensor.matmul(
                            zv, mlp_tm[:, tt, mt * 128:(mt + 1) * 128],
                            sel[:, tt, :], start=(tt == 0), stop=(tt == 3))
                    nc.scalar.activation(myb[:, mt, :], zv, AF.Identity,
                                         bias=mlpb2[:, mt:mt + 1])
                # extend child axis for cyclic windows
                nc.vector.tensor_copy(mlpD[:, :, L:DBL], mlpD[:, :, 0:DBL - L])

              # ---------- pairwise scores ----------
              with (
                  tc.tile_pool(name="pw", bufs=3) as pw,
                  tc.tile_pool(name="psP", bufs=2, space="PSUM") as psP,
              ):
                # groups of 4 slots: broadcast-adds on DVE (mt 0-2) and the
                # idle GpSimd engine (mt 3), one batched tanh ACTIVATE per
                # group, then a burst of 16 matmuls; groups pipeline.
                for g0 in range(0, HPC, 4):
                    S = pw.tile([128, 16, WLEN], F16, tag="S", bufs=3,
                                name=f"S{g0}")
                    for r in range(4):
                        j = g0 + r
                        for mt in range(4):
                            eng = nc.gpsimd if mt == 3 else nc.vector
                            eng.tensor_scalar_add(
                                S[:, 4 * r + mt, :],
                                mlpD[:, mt, 8 * j:8 * j + WLEN],
                                myb[:, mt, j:j + 1])
                    T = pw.tile([128, 16, WLEN], F16, tag="T", bufs=3,
                                name=f"T{g0}")
                    nc.scalar.activation(T[:], S[:], AF.Tanh)
                    sp4 = psP.tile([128, 4, L], F32, tag="sp",
                                   name=f"sp{g0}")
                    for r in range(4):
                        for mt in range(4):
                            nc.tensor.matmul(sp4[:, r, 0:WLEN],
                                             outw[:, mt, :],
                                             T[:, 4 * r + mt, :],
                                             start=(mt == 0),
                                             stop=(mt == 3))
                    stage = pw.tile([1, 4, WLEN], F32, tag="stage",
                                    bufs=2, name=f"stg{g0}")
                    nc.vector.tensor_copy(stage[0:1, :, :],
                                          sp4[0:1, :, 0:WLEN])
                    nc.sync.dma_start(out_d[g0:g0 + 4, :],
                                      stage[0:1, :, :])

    nc.compile()
    return nc


def _packT(W, ktiles, pad_to=None, bias=None):
    WT = np.ascontiguousarray(np.asarray(W).T.astype(np.float32))
    k, n = WT.shape
    if pad_to is not None and k < pad_to:
        WT = np.vstack([WT, np.zeros((pad_to - k, n), np.float32)])
        if bias is not None:
            WT[k, :] = bias
    return np.ascontiguousarray(
        WT.reshape(ktiles, 128, n).transpose(1, 0, 2).astype(np.float16))


def _wrap16(idx):
    a = np.asarray(idx).astype(np.int64).reshape(32, 16).T.astype(np.int16)
    return np.ascontiguousarray(np.tile(a, (8, 1)))


def _splitw(w):
    return np.ascontiguousarray(
        np.stack([w[:, :, :NG // 2], w[:, :, NG // 2:]], axis=0))


def _packow(w):
    ow = np.zeros((128, 4, 128), np.float16)
    ow[:, :, 0] = _bpack(w, 4)
    return np.ascontiguousarray(ow)


def _bpack(b, tiles):
    return np.ascontiguousarray(np.asarray(b, np.float32).reshape(tiles, 128).T)


def kernel(**inputs):
    if "nc" not in _CACHE:
        _CACHE["nc"] = _build_program()
    nc = _CACHE["nc"]

    inp = {k: np.asarray(v) for k, v in inputs.items()}
    widx = inp["word_idx"].astype(np.int64)
    pidx = inp["pos_idx"].astype(np.int64)

    base = {
        "w2": np.ascontiguousarray(
            inp["w_embed"].astype(np.float32).reshape(V2, 2 * WD)),
        "pemb": np.ascontiguousarray(inp["p_embed"].astype(np.float32)),
        "mlpb2": _bpack(2.0 * inp["mlp_b"], 4),
        "outw": _packow(inp["out_w"]),
    }

    def dir_inputs(rev):
        rev = int(rev)
        w = widx[::-1] if rev else widx
        p = pidx[::-1] if rev else pidx
        sfx = "r" if rev else ""
        return {
            "widx": _wrap16(w // 2),
            "pidx": _wrap16(p),
            "wpar": np.ascontiguousarray(
                (w % 2).astype(np.float32).reshape(4, 128).T.reshape(128, 4, 1)),
            "wih0": _packT(inp[f"W_ih_l0{sfx}"], 3, pad_to=384,
                           bias=inp[f"b_ih_l0{sfx}"] + inp[f"b_hh_l0{sfx}"]),
            "whh0": _packT(inp[f"W_hh_l0{sfx}"], 4),
            "wih1loc": _splitw(_packT(inp[f"W_ih_l1{sfx}"], 8)[:, (4 * rev):(4 * rev) + 4, :]),
            "wih1rem": _splitw(_packT(inp[f"W_ih_l1{sfx}"], 8)[:, (4 - 4 * rev):(8 - 4 * rev), :]),
            "whh1": _packT(inp[f"W_hh_l1{sfx}"], 4),
            "b1": _bpack(inp[f"b_ih_l1{sfx}"] + inp[f"b_hh_l1{sfx}"], 16),
            "mlpwloc": np.ascontiguousarray(
                _packT(inp["mlp_W"], 8)[:, (4 * rev):(4 * rev) + 4, :]),
            "mlpwrem": np.ascontiguousarray(
                _packT(inp["mlp_W"], 8)[:, (4 - 4 * rev):(8 - 4 * rev), :]),
            "maskI": np.full((128, 1), 0.0 if rev else 1.0, np.float32),
            "maskR": np.full((128, 1), 1.0 if rev else 0.0, np.float32),
        }

    fwd_in, bwd_in = dir_inputs(False), dir_inputs(True)

    in_maps = []
    for c in range(NCORES):
        rev = c >= 4
        m = dict(base)
        m.update(bwd_in if rev else fwd_in)
        sel = np.zeros((L, HPC), np.float32)
        for j in range(HPC):
            h = _head_of(c, j)
            t = (L - 1 - h) if rev else h          # head row in core time
            sel[t, j] = 1.0
        m["sel"] = np.ascontiguousarray(
            sel.reshape(4, 128, HPC).transpose(1, 0, 2).astype(np.float16))
        in_maps.append(m)

    res = run_bass_kernel_spmd(nc, in_maps, list(range(NCORES)))
    outb = np.float32(inp["out_b"])
    scores2 = np.zeros((L, L), np.float32)
    for c in range(NCORES):
        o = res.results[c]["out"].astype(np.float32) + outb   # [HPC, WLEN]
        for j in range(HPC):
            h = _head_of(c, j)
            b = _child_of_vec(c, j)
            scores2[h, b] = o[j]
            scores2[b, h] = o[j]
    return np.ascontiguousarray(scores2[:, 1:])


# revision 4
# speedup vs baseline: 1.3691x; 1.3691x over previous
"""Trainium2 Bass kernel for nn_DependencyParser.

SPMD over 8 NeuronCores; cores 0-3 run the forward LSTM direction, cores 4-7
the backward direction -- one identical program, direction expressed purely
through per-core DATA (time-reversed gather indices, direction-specific
weights, 0/1 orientation masks).  Per core:
  - on-device embedding gather (dma_gather; paired-row trick since the 50000
    vocab exceeds the int16 index range), PE-transpose to [feature, time]
  - 2-layer LSTM via block Gauss-Seidel sweeps over the hidden sequence;
    each sweep is a batched [2048,512]x[512,512] matmul + gate activations;
    the cell state is computed EXACTLY per sweep with the hardware linear
    scan op (tensor_tensor_scan).  Every wave is processed in two 256-col
    TIME CHUNKS: the scan chains across chunks via an AP initial value, and
    the next wave's dependent matmuls are issued per (kt, chunk) in oldest-
    dependency-first order, so the serial scan->tanh->mult tail is hidden
    under independent matmul work and the PE never idles long enough for
    the HAM clock gate to re-throttle.
  - after each layer the two directions exchange hidden states with TWO
    pairwise AllGathers (h01 fired right after wave 0 of the final sweep,
    h23 after wave 1) so the collective latency overlaps the final sweep
    tail + the next layer's local input pass.
  - head MLP, then pairwise scores tanh(mlp[i]+mlp[j]) @ out_w for 64 head
    "slots".  The pair function is symmetric in (head, child), so each head
    only computes a 264-wide cyclic window of children (mirror coverage
    halves the tanh work); the host reconstructs the full [L, L-1] matrix.
    tanh is batched 4 slots per ACTIVATE; the mt=3 broadcast-adds run on
    the otherwise-idle GpSimd engine.
Matmul operands are fp16 (1 cyc/row); PRE = W_ih@x + b is stored bf16; the
g/o gate PRE is injected into the gate PSUM via identity-matmuls (TensorE),
the i/f gate PRE via DVE adds, balancing PE vs DVE per wave.  Gate tiles are
fp16 so the cell-state chain runs in DVE 2x mode.  L0's LSTM bias rides a
ones-row in the x matmul.  Bulk weights for layer 1 / MLP are DMA'd after
the L0 sweeps start so the embedding gather gets full HBM bandwidth.
"""
import sys
sys.path.insert(0, '/opt/trn_rl_repo')
import numpy as np

import concourse.bass as bass
import concourse.mybir as mybir
import concourse.tile as tile
from concourse import bacc
from concourse.masks import make_identity
from concourse.bass_utils import run_bass_kernel_spmd

F32 = mybir.dt.float32
F16 = mybir.dt.float16
BF16 = mybir.dt.bfloat16
I16 = mybir.dt.int16
AF = mybir.ActivationFunctionType
OP = mybir.AluOpType

L = 512
NG = 2048
V2 = 25000
WD, PD = 256, 64
DIN = WD + PD
M = 512
NCORES = 8
HPC = L // NCORES
WLEN = 264           # pairwise child window (cyclic, via doubled child axis)
DBL = 768            # doubled child axis length actually needed (8*63+264)
K0 = 7
K1 = 8

CHS = ((0, 256), (256, 512))        # time chunks within a wave
INJECT = {4, 5, 6, 7}               # g/o pairs: pre via TensorE identity-matmul

_CACHE = {}


def _head_of(c, j):
    if c < 4:
        return 8 * j + 4 + c
    return 8 * ((31 - j) % 64) + (c - 4)


def _child_of_vec(c, j):
    i = np.arange(WLEN)
    if c < 4:
        return (8 * j + i) % 512
    return (511 - 8 * j - i) % 512


def _emit_sweeps(nc, wp, gp, psum, identr, pre, whh, hA, hB, n_sweeps,
                 on_final=None):
    """Block Gauss-Seidel sweeps, chunked in time.  hA/hB are (tile01,
    tile23) double buffers holding the SHIFTED hidden sequence
    (storage[t] = h_{t-1}).  Wave 2 (hidden dims 256:512) consumes wave 1's
    fresh h01 from the same sweep.  Returns (fin01, fin23) holding the
    UNSHIFTED final h, written into the retiring hprev buffers; on_final(wv,
    tile) is invoked right after each final-sweep wave's unshifted write so
    the caller can overlap collectives with the remaining wave."""
    sig, tanh = AF.Sigmoid, AF.Tanh
    G = {}
    for g in range(4):
        G[g] = gp.tile([128, 4, L], F16, tag=f"G{g}", name=f"G{g}")

    for s in range(n_sweeps):
        hprev, hnew = (hA, hB) if s % 2 == 0 else (hB, hA)
        last = s == n_sweeps - 1
        for wv in range(2):
            hs = 2 * wv
            # pair order i, g, f, o within each chunk: the o gate finishes
            # last so the cell chain completes during the o matmuls.
            pairs = (0, 4, 2, 6) if wv == 0 else (1, 5, 3, 7)
            # oldest-dependency kt first: wave0's freshest input is the
            # previous wave's h23; wave1's is this sweep's fresh h01.
            ktorder = (0, 1, 2, 3) if wv == 0 else (2, 3, 0, 1)
            zps = {}
            if s > 0:
                for a in pairs:
                    zps[a] = psum.tile([128, 2, L], F32, tag="zp",
                                       name=f"zp{s}_{a}")
            u = wp.tile([128, 2, L], F16, tag="u", name=f"u{s}_{wv}")
            cst = wp.tile([128, 2, L], F32, tag="c", name=f"c{s}_{wv}")
            tc_ = wp.tile([128, 2, L], F16, tag="tc", name=f"tc{s}_{wv}")
            for ci, (c0, c1) in enumerate(CHS):
                if s > 0:
                    for kt in ktorder:
                        src = hnew if (wv == 1 and kt < 2) else hprev
                        rhs = src[kt // 2][:, kt % 2, c0:c1]
                        for a in pairs:
                            for half in range(2):
                                nt = 2 * a + half
                                nc.tensor.matmul(
                                    zps[a][:, half, c0:c1],
                                    whh[:, kt, nt * 128:(nt + 1) * 128],
                                    rhs, start=(kt == ktorder[0]),
                                    stop=(kt == ktorder[3] and
                                          a not in INJECT),
                                    skip_group_check=(ci == 1))
                    for a in pairs:
                        if a in INJECT:
                            for half in range(2):
                                nt = 2 * a + half
                                nc.tensor.matmul(zps[a][:, half, c0:c1],
                                                 identr[:],
                                                 pre[:, nt, c0:c1],
                                                 start=False, stop=True,
                                                 skip_group_check=(ci == 1))
                # gate activations for this chunk (i, g, f, o)
                for a in pairs:
                    gate = (2 * a) // 4
                    fn = tanh if gate == 2 else sig
                    if s == 0:
                        nc.scalar.activation(G[gate][:, hs:hs + 2, c0:c1],
                                             pre[:, 2 * a:2 * a + 2, c0:c1],
                                             fn)
                    else:
                        if a not in INJECT:
                            nc.vector.tensor_tensor(
                                zps[a][:, :, c0:c1], zps[a][:, :, c0:c1],
                                pre[:, 2 * a:2 * a + 2, c0:c1], OP.add)
                        nc.scalar.activation(G[gate][:, hs:hs + 2, c0:c1],
                                             zps[a][:, :, c0:c1], fn)
                # cell chain for this chunk
                nc.vector.tensor_tensor(u[:, :, c0:c1],
                                        G[0][:, hs:hs + 2, c0:c1],
                                        G[2][:, hs:hs + 2, c0:c1], OP.mult)
                for b in range(2):
                    init = 0.0 if ci == 0 else cst[:, b, c0 - 1:c0]
                    nc.vector.tensor_tensor_scan(
                        cst[:, b, c0:c1], G[1][:, hs + b, c0:c1],
                        u[:, b, c0:c1], init, OP.mult, OP.add)
                nc.scalar.activation(tc_[:, :, c0:c1], cst[:, :, c0:c1], tanh)
                # shifted store (hnew[t+1] = h_t); the last column drops out
                se = min(c1, L - 1)
                if not last or wv == 0:
                    nc.vector.tensor_tensor(hnew[wv][:, :, c0 + 1:se + 1],
                                            G[3][:, hs:hs + 2, c0:se],
                                            tc_[:, :, c0:se], OP.mult)
                if last:
                    nc.vector.tensor_tensor(hprev[wv][:, :, c0:c1],
                                            G[3][:, hs:hs + 2, c0:c1],
                                            tc_[:, :, c0:c1], OP.mult)
            if last and on_final is not None:
                on_final(wv, hprev[wv])
    return hA if (n_sweeps - 1) % 2 == 0 else hB


def _build_program():
    nc = bacc.Bacc("TRN2", target_bir_lowering=False, debug=False,
                   num_devices=NCORES)

    def dram_in(name, shape, dtype=F32):
        return nc.dram_tensor(name, shape, dtype, kind="ExternalInput")

    w2_d = dram_in("w2", [V2, 2 * WD])
    pemb_d = dram_in("pemb", [50, PD])
    widx_d = dram_in("widx", [128, 32], I16)
    pidx_d = dram_in("pidx", [128, 32], I16)
    wpar_d = dram_in("wpar", [128, 4, 1])
    wih0_d = dram_in("wih0", [128, 3, NG], F16)   # bias folded in row 320
    whh0_d = dram_in("whh0", [128, 4, NG], F16)
    wihl_d = dram_in("wih1loc", [2, 128, 4, NG // 2], F16)
    wihr_d = dram_in("wih1rem", [2, 128, 4, NG // 2], F16)
    whh1_d = dram_in("whh1", [128, 4, NG], F16)
    b1_d = dram_in("b1", [128, 16])
    mlpwl_d = dram_in("mlpwloc", [128, 4, M], F16)
    mlpwr_d = dram_in("mlpwrem", [128, 4, M], F16)
    mlpb2_d = dram_in("mlpb2", [128, 4])
    outw_d = dram_in("outw", [128, 4, 128], F16)
    sel_d = dram_in("sel", [128, 4, HPC], F16)
    mi_d = dram_in("maskI", [128, 1])
    mr_d = dram_in("maskR", [128, 1])
    out_d = nc.dram_tensor("out", [HPC, WLEN], F32, kind="ExternalOutput")

    GROUPS = [[0, 4], [1, 5], [2, 6], [3, 7]]

    with tile.TileContext(nc) as tc:
        with (
            tc.tile_pool(name="pp", bufs=1) as pp,
            tc.tile_pool(name="wp", bufs=2) as wp,
            tc.tile_pool(name="gp", bufs=1) as gp,
            tc.tile_pool(name="dram", bufs=2, space="DRAM") as dp,
        ):
            ident = pp.tile([128, 128], F32, tag="ident")
            make_identity(nc, ident[:])
            identr = pp.tile([128, 128], BF16, tag="identr")
            nc.vector.tensor_copy(identr[:], ident[:])
            identh = pp.tile([128, 128], F16, tag="identh")
            nc.vector.tensor_copy(identh[:], ident[:])
            zsrc = pp.tile([128, 1], F32, tag="zsrc")
            nc.vector.memset(zsrc[:], 0.0)
            mI = pp.tile([128, 1], F32, tag="mI")
            mR = pp.tile([128, 1], F32, tag="mR")
            nc.sync.dma_start(mI[:], mi_d[:])
            nc.sync.dma_start(mR[:], mr_d[:])
            xrem = pp.tile([128, 4, L], F16, tag="xrem")
            mlpwl = pp.tile([128, 4, M], F16, tag="mlpwl")
            mlpwr = pp.tile([128, 4, M], F16, tag="mlpwr")
            sel = pp.tile([128, 4, HPC], F16, tag="sel")
            mlpb2 = pp.tile([128, 4], F32, tag="mlpb2")
            outw = pp.tile([128, 4, 128], F16, tag="outw")
            hA = tuple(pp.tile([128, 2, L], F16, tag=f"hA{i}", name=f"hA{i}")
                       for i in range(2))
            hB = tuple(pp.tile([128, 2, L], F16, tag=f"hB{i}", name=f"hB{i}")
                       for i in range(2))
            for t in hA + hB:
                nc.vector.tensor_copy(t[:, :, 0:1],
                                      zsrc[:, 0:1].to_broadcast([128, 2, 1]))
            s0 = pp.tile([128, 4, L], F16, tag="slot0")
            s1 = pp.tile([128, 4, L], F16, tag="slot1")

            def build_xrem_half(slot0, slot1, kts):
                # remote direction's hidden, re-oriented to my time order
                for kt in kts:
                    tmp = wp.tile([128, L], F32, tag="u", name=f"xt{kt}")
                    nc.vector.tensor_scalar_mul(tmp[:], slot0[:, kt, ::-1],
                                                mR[:])
                    nc.vector.scalar_tensor_tensor(
                        xrem[:, kt, :], slot1[:, kt, ::-1], mI[:], tmp[:],
                        OP.mult, OP.add)

            with tc.tile_pool(name="post", bufs=1) as post:
              mlp_tm = post.tile([128, 4, M], F16, tag="mlp_tm")
              mlpD = post.tile([128, 4, DBL], F16, tag="mlpD")
              myb = post.tile([128, 4, HPC], F32, tag="myb")
              with tc.tile_pool(name="psA", bufs=4, space="PSUM") as psum:
                with (
                    tc.tile_pool(name="prep", bufs=1) as prep,
                    tc.tile_pool(name="wts", bufs=1) as wts,
                ):
                    pre0 = prep.tile([128, 16, L], BF16, tag="pre0")
                    pre1 = prep.tile([128, 16, L], BF16, tag="pre1")
                    bT1 = prep.tile([128, 16], F32, tag="bT1")

                    # ---------- gather + transpose ----------
                    # index DMAs + gathers issued FIRST; the bulk layer-1/MLP
                    # weights are deferred until the L0 sweeps are emitted so
                    # the gather gets full HBM bandwidth.
                    xT = prep.tile([128, 3, L], F16, tag="xT")
                    with tc.tile_pool(name="pC", bufs=1) as pC:
                        widx = pC.tile([128, 32], I16, tag="widx")
                        pidx = pC.tile([128, 32], I16, tag="pidx")
                        wpar = pC.tile([128, 4, 1], F32, tag="wpar")
                        nc.sync.dma_start(widx[:], widx_d[:])
                        nc.sync.dma_start(pidx[:], pidx_d[:])
                        nc.sync.dma_start(wpar[:], wpar_d[:])
                        pair = pC.tile([128, 4, 2 * WD], F32, tag="pair")
                        nc.gpsimd.dma_gather(pair[:], w2_d[:], widx[:], L, L,
                                             elem_size=2 * WD)
                        xp = pC.tile([128, 4, PD], F32, tag="xp")
                        nc.gpsimd.dma_gather(xp[:], pemb_d[:], pidx[:], L, L,
                                             elem_size=PD)
                        # layer-0 weights (needed first)
                        wih0 = wts.tile([128, 3, NG], F16, tag="wih0")
                        nc.sync.dma_start(wih0[:], wih0_d[:])
                        whh0 = wts.tile([128, 4, NG], F16, tag="whh0")
                        nc.sync.dma_start(whh0[:], whh0_d[:])
                        x = pC.tile([128, 4, DIN], F32, tag="x")
                        xw = x[:, :, 0:WD]
                        nc.vector.tensor_tensor(xw, pair[:, :, WD:2 * WD],
                                                pair[:, :, 0:WD], OP.subtract)
                        nc.vector.tensor_tensor(
                            xw, xw, wpar[:].to_broadcast([128, 4, WD]),
                            OP.mult)
                        nc.vector.tensor_tensor(xw, xw, pair[:, :, 0:WD],
                                                OP.add)
                        nc.vector.tensor_copy(x[:, :, WD:DIN], xp[:])
                        # zero pad rows, then the ones bias row (din 320)
                        nc.vector.tensor_copy(
                            xT[64:128, 2, :],
                            zsrc[64:128, 0:1].to_broadcast([64, L]))
                        nc.vector.memset(xT[64:65, 2, :], 1.0)
                        for ct in range(4):
                            for dblk, wdt in ((0, 128), (1, 128), (2, 64)):
                                tp = psum.tile([128, 2, L], F32, tag="zp",
                                               name=f"tp{ct}_{dblk}")
                                nc.tensor.transpose(
                                    tp[0:wdt, 0, 0:128],
                                    x[:, ct, dblk * 128:dblk * 128 + wdt],
                                    ident[:])
                                nc.vector.tensor_copy(
                                    xT[0:wdt, dblk, ct * 128:(ct + 1) * 128],
                                    tp[0:wdt, 0, 0:128])

                    # ------- layer 0 pre (bias folded into matmul) -------
                    for a in (0, 4, 2, 6, 1, 5, 3, 7):
                        zp = psum.tile([128, 2, L], F32, tag="zp",
                                       name=f"p0_{a}")
                        for kt in range(3):
                            for half in range(2):
                                nt = 2 * a + half
                                nc.tensor.matmul(
                                    zp[:, half, :],
                                    wih0[:, kt, nt * 128:(nt + 1) * 128],
                                    xT[:, kt, :], start=(kt == 0),
                                    stop=(kt == 2))
                        nc.vector.tensor_copy(pre0[:, 2 * a:2 * a + 2, :],
                                              zp[:])

                    # deferred bulk weights (layer 1 + MLP + pairwise)
                    wl = []
                    wr = []
                    for nh in range(2):
                        t = wts.tile([128, 4, NG // 2], F16, tag=f"wl{nh}")
                        nc.sync.dma_start(t[:], wihl_d[nh])
                        wl.append(t)
                    for nh in range(2):
                        t = wts.tile([128, 4, NG // 2], F16, tag=f"wr{nh}")
                        nc.sync.dma_start(t[:], wihr_d[nh])
                        wr.append(t)
                    whh1 = wts.tile([128, 4, NG], F16, tag="whh1")
                    nc.sync.dma_start(whh1[:], whh1_d[:])
                    nc.sync.dma_start(bT1[:], b1_d[:])
                    nc.sync.dma_start(mlpwl[:], mlpwl_d[:])
                    nc.sync.dma_start(mlpwr[:], mlpwr_d[:])
                    nc.sync.dma_start(sel[:], sel_d[:])
                    nc.sync.dma_start(mlpb2[:], mlpb2_d[:])
                    nc.sync.dma_start(outw[:], outw_d[:])

                    # ---------- layer 0 sweeps (split collective) ----------
                    ag0 = {}

                    def fire_ag0(wv, htile):
                        inb = dp.tile([128, 2, L], F16, tag=f"inb{wv}",
                                      name=f"inb0_{wv}")
                        outb = dp.tile([2, 128, 2, L], F16, tag=f"outb{wv}",
                                       name=f"outb0_{wv}")
                        nc.sync.dma_start(inb[:], htile[:])
                        nc.gpsimd.collective_compute(
                            "AllGather", OP.bypass, replica_groups=GROUPS,
                            ins=[inb.opt()], outs=[outb.opt()])
                        ag0[wv] = outb

                    hfin0 = _emit_sweeps(nc, wp, gp, psum, identr, pre0,
                                         whh0, hA, hB, K0, on_final=fire_ag0)

                    # ------- layer 1 pre: local pass (overlaps collective) --
                    def pass1_pair(a):
                        zp = psum.tile([128, 2, L], F32, tag="zp",
                                       name=f"p1_{a}")
                        nh = (2 * a) // 8
                        for kt in range(4):
                            for half in range(2):
                                nt = 2 * a + half
                                off = (nt - nh * 8) * 128
                                nc.tensor.matmul(
                                    zp[:, half, :],
                                    wl[nh][:, kt, off:off + 128],
                                    hfin0[kt // 2][:, kt % 2, :],
                                    start=(kt == 0), stop=(kt == 3))
                        for half in range(2):
                            nt = 2 * a + half
                            nc.scalar.activation(pre1[:, nt, :],
                                                 zp[:, half, :], AF.Identity,
                                                 bias=bT1[:, nt:nt + 1])

                    for a in (0, 4, 2, 6, 1, 5, 3, 7):
                        pass1_pair(a)
                    # first half of the exchanged hidden -> xrem kt 0/1
                    nc.sync.dma_start(s0[:, 0:2, :], ag0[0][0])
                    nc.sync.dma_start(s1[:, 0:2, :], ag0[0][1])
                    build_xrem_half(s0, s1, (0, 1))
                    # remote pass, accumulated into pre1 via identity-inject;
                    # two batches of 4 pairs, kt 0/1 first so the second
                    # AllGather half has time to land.
                    for batch in ((0, 4, 2, 6), (1, 5, 3, 7)):
                        zpb = {}
                        for a in batch:
                            zpb[a] = psum.tile([128, 2, L], F32, tag="zp",
                                               name=f"p2_{a}")
                            nh = (2 * a) // 8
                            for half in range(2):
                                nt = 2 * a + half
                                nc.tensor.matmul(zpb[a][:, half, :],
                                                 identr[:], pre1[:, nt, :],
                                                 start=True, stop=False)
                            for kt in (0, 1):
                                for half in range(2):
                                    nt = 2 * a + half
                                    off = (nt - nh * 8) * 128
                                    nc.tensor.matmul(
                                        zpb[a][:, half, :],
                                        wr[nh][:, kt, off:off + 128],
                                        xrem[:, kt, :], start=False,
                                        stop=False)
                        if batch[0] == 0:
                            nc.sync.dma_start(s0[:, 2:4, :], ag0[1][0])
                            nc.sync.dma_start(s1[:, 2:4, :], ag0[1][1])
                            build_xrem_half(s0, s1, (2, 3))
                        for a in batch:
                            nh = (2 * a) // 8
                            for kt in (2, 3):
                                for half in range(2):
                                    nt = 2 * a + half
                                    off = (nt - nh * 8) * 128
                                    nc.tensor.matmul(
                                        zpb[a][:, half, :],
                                        wr[nh][:, kt, off:off + 128],
                                        xrem[:, kt, :], start=False,
                                        stop=(kt == 3))
                        for a in batch:
                            nc.scalar.activation(pre1[:, 2 * a:2 * a + 2, :],
                                                 zpb[a][:], AF.Copy)

                    # ------- layer 1 sweeps (reuse the L0 h buffers) -------
                    for t in hfin0:
                        nc.vector.tensor_copy(
                            t[:, :, 0:1],
                            zsrc[:, 0:1].to_broadcast([128, 2, 1]))
                    ag1 = {}

                    def fire_ag1(wv, htile):
                        inb = dp.tile([128, 2, L], F16, tag=f"inb{wv}",
                                      name=f"inb1_{wv}")
                        outb = dp.tile([2, 128, 2, L], F16, tag=f"outb{wv}",
                                       name=f"outb1_{wv}")
                        nc.sync.dma_start(inb[:], htile[:])
                        nc.gpsimd.collective_compute(
                            "AllGather", OP.bypass, replica_groups=GROUPS,
                            ins=[inb.opt()], outs=[outb.opt()])
                        ag1[wv] = outb

                    hfin1 = _emit_sweeps(nc, wp, gp, psum, identr, pre1,
                                         whh1, hA, hB, K1, on_final=fire_ag1)

                # ---------- MLP (into mlpD[:, :, 0:L] and mlp_tm) ----------
                for mp in (0, 2):
                    zp = psum.tile([128, 2, L], F32, tag="zp", name=f"mm{mp}")
                    for kt in range(4):
                        for half in range(2):
                            mt = mp + half
                            nc.tensor.matmul(
                                zp[:, half, :],
                                mlpwl[:, kt, mt * 128:(mt + 1) * 128],
                                hfin1[kt // 2][:, kt % 2, :],
                                start=(kt == 0), stop=(kt == 3))
                    nc.scalar.activation(mlpD[:, mp:mp + 2, 0:L], zp[:],
                                         AF.Copy)
                for tq in (0, 2):
                    zp = psum.tile([128, 2, L], F32, tag="zp", name=f"mt{tq}")
                    for kt in range(4):
                        for half in range(2):
                            tt = tq + half
                            nc.tensor.matmul(
                                zp[:, half, :],
                                hfin1[kt // 2][:, kt % 2, tt * 128:(tt + 1) * 128],
                                mlpwl[:, kt, :], start=(kt == 0),
                                stop=(kt == 3))
                    nc.scalar.activation(mlp_tm[:, tq:tq + 2, :], zp[:],
                                         AF.Copy)
                nc.sync.dma_start(s0[:, 0:2, :], ag1[0][0])
                nc.sync.dma_start(s1[:, 0:2, :], ag1[0][1])
                build_xrem_half(s0, s1, (0, 1))
                # remote halves accumulated via identity-inject; kt 0/1
                # phase first across all four groups, then kt 2/3.
                zr = {}
                for mp in (0, 2):
                    zr[('m', mp)] = psum.tile([128, 2, L], F32, tag="zp",
                                              name=f"rm{mp}")
                    zr[('t', mp)] = psum.tile([128, 2, L], F32, tag="zp",
                                              name=f"rt{mp}")
                for mp in (0, 2):
                    zp = zr[('m', mp)]
                    for half in range(2):
                        mt = mp + half
                        nc.tensor.matmul(zp[:, half, :], identh[:],
                                         mlpD[:, mt, 0:L], start=True,
                                         stop=False)
                    for kt in (0, 1):
                        for half in range(2):
                            mt = mp + half
                            nc.tensor.matmul(
                                zp[:, half, :],
                                mlpwr[:, kt, mt * 128:(mt + 1) * 128],
                                xrem[:, kt, :], start=False, stop=False)
                for tq in (0, 2):
                    zp = zr[('t', tq)]
                    for half in range(2):
                        tt = tq + half
                        nc.tensor.matmul(zp[:, half, :], identh[:],
                                         mlp_tm[:, tt, :], start=True,
                                         stop=False)
                    for kt in (0, 1):
                        for half in range(2):
                            tt = tq + half
                            nc.tensor.matmul(
                                zp[:, half, :],
                                xrem[:, kt, tt * 128:(tt + 1) * 128],
                                mlpwr[:, kt, :], start=False, stop=False)
                nc.sync.dma_start(s0[:, 2:4, :], ag1[1][0])
                nc.sync.dma_start(s1[:, 2:4, :], ag1[1][1])
                build_xrem_half(s0, s1, (2, 3))
                for mp in (0, 2):
                    zp = zr[('m', mp)]
                    for kt in (2, 3):
                        for half in range(2):
                            mt = mp + half
                            nc.tensor.matmul(
                                zp[:, half, :],
                                mlpwr[:, kt, mt * 128:(mt + 1) * 128],
                                xrem[:, kt, :], start=False, stop=(kt == 3))
                    nc.scalar.activation(mlpD[:, mp:mp + 2, 0:L], zp[:],
                                         AF.Copy)
                for tq in (0, 2):
                    zp = zr[('t', tq)]
                    for kt in (2, 3):
                        for half in range(2):
                            tt = tq + half
                            nc.tensor.matmul(
                                zp[:, half, :],
                                xrem[:, kt, tt * 128:(tt + 1) * 128],
                                mlpwr[:, kt, :], start=False, stop=(kt == 3))
                    nc.scalar.activation(mlp_tm[:, tq:tq + 2, :], zp[:],
                                         AF.Copy)
                # head-slot mlp vectors (one-hot select over time) + 2*bias
                for mt in range(4):
                    zp = psum.tile([128, 2, L], F32, tag="zp", name=f"my{mt}")
                    zv = zp[:, 0, 0:HPC]
                    for tt in range(4):
                        nc.tensor.matmul(
                            zv, mlp_tm[:, tt, mt * 128:(mt + 1) * 128],
                            sel[:, tt, :], start=(tt == 0), stop=(tt == 3))
                    nc.scalar.activation(myb[:, mt, :], zv, AF.Identity,
                                         bias=mlpb2[:, mt:mt + 1])
                # extend child axis for cyclic windows
                nc.vector.tensor_copy(mlpD[:, :, L:DBL], mlpD[:, :, 0:DBL - L])

              # ---------- pairwise scores ----------
              with (
                  tc.tile_pool(name="pw", bufs=3) as pw,
                  tc.tile_pool(name="psP", bufs=2, space="PSUM") as psP,
              ):
                # groups of 4 slots: broadcast-adds on DVE (mt 0-2) and the
                # idle GpSimd engine (mt 3), one batched tanh ACTIVATE per
                # group, then a burst of 16 matmuls; groups pipeline.
                for g0 in range(0, HPC, 4):
                    S = pw.tile([128, 16, WLEN], F16, tag="S", bufs=3,
                                name=f"S{g0}")
                    for r in range(4):
                        j = g0 + r
                        for mt in range(4):
                            nc.vector.tensor_scalar_add(
                                S[:, 4 * r + mt, :],
                                mlpD[:, mt, 8 * j:8 * j + WLEN],
                                myb[:, mt, j:j + 1])
                    T = pw.tile([128, 16, WLEN], F16, tag="T", bufs=3,
                                name=f"T{g0}")
                    nc.scalar.activation(T[:], S[:], AF.Tanh)
                    sp4 = psP.tile([128, 4, L], F32, tag="sp",
                                   name=f"sp{g0}")
                    for r in range(4):
                        for mt in range(4):
                            nc.tensor.matmul(sp4[:, r, 0:WLEN],
                                             outw[:, mt, :],
                                             T[:, 4 * r + mt, :],
                                             start=(mt == 0),
                                             stop=(mt == 3))
                    stage = pw.tile([1, 4, WLEN], F32, tag="stage",
                                    bufs=2, name=f"stg{g0}")
                    nc.vector.tensor_copy(stage[0:1, :, :],
                                          sp4[0:1, :, 0:WLEN])
                    nc.sync.dma_start(out_d[g0:g0 + 4, :],
                                      stage[0:1, :, :])

    nc.compile()
    return nc


def _packT(W, ktiles, pad_to=None, bias=None):
    WT = np.ascontiguousarray(np.asarray(W).T.astype(np.float32))
    k, n = WT.shape
    if pad_to is not None and k < pad_to:
        WT = np.vstack([WT, np.zeros((pad_to - k, n), np.float32)])
        if bias is not None:
            WT[k, :] = bias
    return np.ascontiguousarray(
        WT.reshape(ktiles, 128, n).transpose(1, 0, 2).astype(np.float16))


def _wrap16(idx):
    a = np.asarray(idx).astype(np.int64).reshape(32, 16).T.astype(np.int16)
    return np.ascontiguousarray(np.tile(a, (8, 1)))


def _splitw(w):
    return np.ascontiguousarray(
        np.stack([w[:, :, :NG // 2], w[:, :, NG // 2:]], axis=0))


def _packow(w):
    ow = np.zeros((128, 4, 128), np.float16)
    ow[:, :, 0] = _bpack(w, 4)
    return np.ascontiguousarray(ow)


def _bpack(b, tiles):
    return np.ascontiguousarray(np.asarray(b, np.float32).reshape(tiles, 128).T)


def kernel(**inputs):
    if "nc" not in _CACHE:
        _CACHE["nc"] = _build_program()
    nc = _CACHE["nc"]

    inp = {k: np.asarray(v) for k, v in inputs.items()}
    widx = inp["word_idx"].astype(np.int64)
    pidx = inp["pos_idx"].astype(np.int64)

    base = {
        "w2": np.ascontiguousarray(
            inp["w_embed"].astype(np.float32).reshape(V2, 2 * WD)),
        "pemb": np.ascontiguousarray(inp["p_embed"].astype(np.float32)),
        "mlpb2": _bpack(2.0 * inp["mlp_b"], 4),
        "outw": _packow(inp["out_w"]),
    }

    def dir_inputs(rev):
        rev = int(rev)
        w = widx[::-1] if rev else widx
        p = pidx[::-1] if rev else pidx
        sfx = "r" if rev else ""
        return {
            "widx": _wrap16(w // 2),
            "pidx": _wrap16(p),
            "wpar": np.ascontiguousarray(
                (w % 2).astype(np.float32).reshape(4, 128).T.reshape(128, 4, 1)),
            "wih0": _packT(inp[f"W_ih_l0{sfx}"], 3, pad_to=384,
                           bias=inp[f"b_ih_l0{sfx}"] + inp[f"b_hh_l0{sfx}"]),
            "whh0": _packT(inp[f"W_hh_l0{sfx}"], 4),
            "wih1loc": _splitw(_packT(inp[f"W_ih_l1{sfx}"], 8)[:, (4 * rev):(4 * rev) + 4, :]),
            "wih1rem": _splitw(_packT(inp[f"W_ih_l1{sfx}"], 8)[:, (4 - 4 * rev):(8 - 4 * rev), :]),
            "whh1": _packT(inp[f"W_hh_l1{sfx}"], 4),
            "b1": _bpack(inp[f"b_ih_l1{sfx}"] + inp[f"b_hh_l1{sfx}"], 16),
            "mlpwloc": np.ascontiguousarray(
                _packT(inp["mlp_W"], 8)[:, (4 * rev):(4 * rev) + 4, :]),
            "mlpwrem": np.ascontiguousarray(
                _packT(inp["mlp_W"], 8)[:, (4 - 4 * rev):(8 - 4 * rev), :]),
            "maskI": np.full((128, 1), 0.0 if rev else 1.0, np.float32),
            "maskR": np.full((128, 1), 1.0 if rev else 0.0, np.float32),
        }

    fwd_in, bwd_in = dir_inputs(False), dir_inputs(True)

    in_maps = []
    for c in range(NCORES):
        rev = c >= 4
        m = dict(base)
        m.update(bwd_in if rev else fwd_in)
        sel = np.zeros((L, HPC), np.float32)
        for j in range(HPC):
            h = _head_of(c, j)
            t = (L - 1 - h) if rev else h          # head row in core time
            sel[t, j] = 1.0
        m["sel"] = np.ascontiguousarray(
            sel.reshape(4, 128, HPC).transpose(1, 0, 2).astype(np.float16))
        in_maps.append(m)

    res = run_bass_kernel_spmd(nc, in_maps, list(range(NCORES)))
    outb = np.float32(inp["out_b"])
    scores2 = np.zeros((L, L), np.float32)
    for c in range(NCORES):
        o = res.results[c]["out"].astype(np.float32) + outb   # [HPC, WLEN]
        for j in range(HPC):
            h = _head_of(c, j)
            b = _child_of_vec(c, j)
            scores2[h, b] = o[j]
            scores2[b, h] = o[j]
    return np.ascontiguousarray(scores2[:, 1:])
